# revision 1
# baseline (speedup 1.0000x reference)
"""Trainium2 Bass kernel for nn_Attention_64235530879146.

Reference computation (per batch element, C=512, T=H*W=1024, 32 groups,
8 heads of ch=64):
    xn = GroupNorm(x) * gn_weight + gn_bias          # [C, T]
    qkv = W1 @ xn + b1                               # [3C, T]
    per head: St[s,t] = (k*sc)^T (q*sc),  sc = ch**-0.25
              Wt = exp(St)   (no max subtraction; |S| < 8 for N(0,1) inputs,
                              far inside fp32 exp range)
              a[c,t] = sum_s v[c,s] Wt[s,t] / r[t],  r[t] = sum_s Wt[s,t]
    out = a + x

Sharding: pure data-parallel over batch - 8 batch elements on 8 NeuronCores,
no collectives. Per-core layout keeps C (or s) on SBUF partitions so every
matmul contracts over the partition dim:
  - GroupNorm: bn_stats/bn_aggr per channel, 16-channel group reduction and
    the broadcast back both via tiny PE matmuls with indicator matrices.
  - QKV: lhsT = W1^T chunk (host-transposed), rhs = xn -> q,k in [o, t]
    layout; v is produced directly TRANSPOSED (lhsT = xn chunk, rhs = W1v^T)
    so attention needs no on-chip transposes at all.
  - scores: lhsT = k[64, 128-chunk], rhs = q[64, 512] -> St[s, t] in PSUM;
    ACT Exp PSUM->SBUF (bf16).
  - AV: lhsT = vT_aug[128, 65] (v^T chunk + ones column), rhs = Wt[128, 512],
    accumulated over the 8 s-tiles -> PSUM rows 0:64 = a, row 64 = r. The
    ones column makes the AV matmul emit the softmax denominator for free.
  - Per head, scores+exp (pass A) fill 8 buffered Wt tiles, then the 16 AV
    matmuls (pass B) run dependency-free; pass B of head h overlaps pass A
    of head h+1 so the PE stays dense while ACT (exp) is the limiter.
  - Epilogue: PSUM evacuated immediately (a -> o_st, r -> rrow on DVE); the
    reciprocal runs off the critical path: DMA-reshape r to [128, 8] (all-
    lane DVE reciprocal), DMA back to a row, DMA row-broadcast to the 64
    channel partitions; out = a * (1/r) + x, DMA per head to DRAM.

Matmul inputs are bf16 (fp32 PSUM accumulate): measured end-to-end relative
error vs an fp64 reference is ~3.5e-4. Weights are transposed/reformatted on
the host in _make_in_maps (pure layout prep, no arithmetic beyond a bf16
cast).
"""
import numpy as np

GROUPS = 32
HEADS = 8
EPS = 1e-5
C = 512
T = 1024
CH = C // HEADS            # 64
SCALE = float(CH) ** -0.25
N_CORES = 8



def _build_nc(debug_taps=False):
    import concourse.bass as bass
    import concourse.mybir as mybir
    import concourse.tile as tile
    from concourse import bacc

    f32 = mybir.dt.float32
    f32r = mybir.dt.float32r
    bf16 = mybir.dt.bfloat16
    Alu = mybir.AluOpType
    Act = mybir.ActivationFunctionType

    nc = bacc.Bacc("TRN2", target_bir_lowering=False, debug=False)

    x_d = nc.declare_dram_parameter("x", [C, T], f32, isOutput=False)
    w1t_d = nc.declare_dram_parameter("w1t", [C, 3 * C], mybir.dt.bfloat16, isOutput=False)
    b1r_d = nc.declare_dram_parameter("b1r", [128, 12], f32, isOutput=False)
    b1v_d = nc.declare_dram_parameter("b1v", [1, C], f32, isOutput=False)
    gnw_d = nc.declare_dram_parameter("gnw", [128, 4], f32, isOutput=False)
    gnb_d = nc.declare_dram_parameter("gnb", [128, 4], f32, isOutput=False)
    ind16_d = nc.declare_dram_parameter("ind16", [128, 8], f32, isOutput=False)
    indT_d = nc.declare_dram_parameter("indT", [8, 128], f32, isOutput=False)
    out_d = nc.declare_dram_parameter("out", [C, T], f32, isOutput=True)
    if debug_taps:
        dbg_xn = nc.declare_dram_parameter("dbg_xn", [128, 4, T], f32, isOutput=True)
        dbg_q = nc.declare_dram_parameter("dbg_q", [128, 4, T], f32, isOutput=True)
        dbg_k = nc.declare_dram_parameter("dbg_k", [128, 4, T], f32, isOutput=True)
        dbg_vt = nc.declare_dram_parameter("dbg_vt", [128, 8, 8, 65], f32, isOutput=True)
        dbg_wt = nc.declare_dram_parameter("dbg_wt", [128, T], f32, isOutput=True)
        dbg_av = nc.declare_dram_parameter("dbg_av", [128, T], f32, isOutput=True)
        dbg_r = nc.declare_dram_parameter("dbg_r", [128, T], f32, isOutput=True)

    with tile.TileContext(nc) as tc:
        with (
            tc.tile_pool(name="const", bufs=1) as cst,
            tc.tile_pool(name="work", bufs=2) as work,
            tc.tile_pool(name="wtp", bufs=4) as wtp,
            tc.tile_pool(name="outp", bufs=3) as outp,
            tc.tile_pool(name="ps", bufs=2, space="PSUM") as ps,
        ):
            # ---------------- loads ----------------
            # x split into 4 c-tiles on the SP queue so GroupNorm stats can
            # start as soon as each tile lands; w1t on the ACT queue and the
            # residual copy / small constants on the GpSimd queue so the three
            # big loads stream in parallel.
            xv = x_d.ap().rearrange("(i p) t -> i p t", p=128)
            x_sb = cst.tile([128, 4, T], f32)
            for i in range(4):
                nc.sync.dma_start(out=x_sb[:, i, :], in_=xv[i])
            w1t_sb = cst.tile([128, 4, 3 * C], bf16)
            w1tv = w1t_d.ap().rearrange("(i p) o -> p i o", p=128)
            nc.scalar.dma_start(out=w1t_sb[:, :, 2 * C :], in_=w1tv[:, :, 2 * C :])
            nc.scalar.dma_start(out=w1t_sb[:, :, : 2 * C], in_=w1tv[:, :, : 2 * C])
            b1r_sb = cst.tile([128, 12], f32)
            nc.gpsimd.dma_start(out=b1r_sb, in_=b1r_d[:, :])
            gnw_sb = cst.tile([128, 4], f32)
            nc.gpsimd.dma_start(out=gnw_sb, in_=gnw_d[:, :])
            gnb_sb = cst.tile([128, 4], f32)
            nc.gpsimd.dma_start(out=gnb_sb, in_=gnb_d[:, :])
            b1v_bc = cst.tile([128, C], f32)
            nc.gpsimd.dma_start(out=b1v_bc, in_=b1v_d.ap().to_broadcast((128, C)))

            # group indicator constants (from host): ind16[c,g]=1/16, indT[g,c]=1
            ind16 = cst.tile([128, 8], f32)
            nc.gpsimd.dma_start(out=ind16, in_=ind16_d[:, :])
            indT = cst.tile([8, 128], f32)
            nc.gpsimd.dma_start(out=indT, in_=indT_d[:, :])
            eps8 = cst.tile([8, 1], f32)
            nc.vector.memset(eps8, EPS)

            # ---------------- GroupNorm stats ----------------
            # per-channel bn stats -> [mean, var, mean^2] per 128-channel tile
            rhs3 = cst.tile([128, 4, 3], f32)
            for i in range(4):
                st6 = work.tile([128, 2, 6], f32, tag="st6")
                nc.vector.bn_stats(out=st6[:, 0, :], in_=x_sb[:, i, 0:512])
                nc.vector.bn_stats(out=st6[:, 1, :], in_=x_sb[:, i, 512:1024])
                mv = work.tile([128, 2], f32, tag="mv")
                nc.vector.bn_aggr(out=mv, in_=st6)
                nc.vector.tensor_copy(out=rhs3[:, i, 0:2], in_=mv)
                nc.vector.tensor_mul(rhs3[:, i, 2:3], mv[:, 0:1], mv[:, 0:1])

            # reduce 16-channel groups via PE: [8 groups, (mu, Evar, Emu2)] per tile
            stats_ps = ps.tile([8, 12], f32, tag="av")
            for i in range(4):
                nc.tensor.matmul(
                    out=stats_ps[:, 3 * i : 3 * i + 3],
                    lhsT=ind16,
                    rhs=rhs3[:, i, :],
                    start=True,
                    stop=True,
                )
            sg = cst.tile([8, 12], f32)
            nc.vector.tensor_copy(out=sg, in_=stats_ps)
            # musig[:, 0, i] = mu_g ; musig[:, 1, i] = rstd_g
            musig = cst.tile([8, 2, 4], f32)
            mu_v = sg.rearrange("p (i three) -> p i three", three=3)
            nc.vector.tensor_copy(out=musig[:, 0, :], in_=mu_v[:, :, 0])
            var_g = cst.tile([8, 4], f32)
            nc.vector.tensor_add(var_g, mu_v[:, :, 1], mu_v[:, :, 2])
            mu2 = cst.tile([8, 4], f32)
            nc.vector.tensor_mul(mu2, mu_v[:, :, 0], mu_v[:, :, 0])
            nc.vector.tensor_sub(var_g, var_g, mu2)
            # rstd = 1/sqrt(var + eps): ACT Sqrt (one table load) + tiny DVE recip
            sdv = cst.tile([8, 4], f32)
            nc.scalar.activation(out=sdv, in_=var_g, func=Act.Sqrt, bias=eps8, scale=1.0)
            nc.vector.reciprocal(out=musig[:, 1, :], in_=sdv)

            # broadcast (mu, rstd) back to channels; fold gn affine:
            # a_c = gnw * rstd ; b_c = gnb - mu * a_c ; xn = x*a_c + b_c
            xn_sb = cst.tile([128, 4, T], bf16)
            af = cst.tile([128, 4, 2], f32)
            for i in range(4):
                musig_ps = ps.tile([128, 2], f32, tag="av")
                nc.tensor.matmul(
                    out=musig_ps, lhsT=indT, rhs=musig[:, :, i], start=True, stop=True
                )
                nc.vector.tensor_mul(af[:, i, 0:1], gnw_sb[:, i : i + 1], musig_ps[:, 1:2])
                tmp = work.tile([128, 1], f32, tag="tmp1")
                nc.vector.tensor_mul(tmp, musig_ps[:, 0:1], af[:, i, 0:1])
                nc.vector.tensor_sub(af[:, i, 1:2], gnb_sb[:, i : i + 1], tmp)
                nc.vector.tensor_scalar(
                    out=xn_sb[:, i, :],
                    in0=x_sb[:, i, :],
                    scalar1=af[:, i, 0:1],
                    scalar2=af[:, i, 1:2],
                    op0=Alu.mult,
                    op1=Alu.add,
                )

            # ---------------- QKV ----------------
            q_sb = cst.tile([128, 4, T], bf16)
            k_sb = cst.tile([128, 4, T], bf16)
            # vT_aug layout per (s_tile, head): v^T at cols 0:64, ones at col 64.
            # The ones column makes the AV matmul also emit the softmax denom r.
            vt_sb = cst.tile([128, 8, 8, 65], bf16)
            nc.vector.tensor_copy(
                out=vt_sb[:, :, :, 64:65],
                in_=nc.const_aps.tensor(1.0, (128, 8, 8, 1), bf16),
            )

            # q / k projections: out rows = 8 o-chunks (4 q + 4 k)
            for j in range(8):
                qk_ps = ps.tile([128, T], f32, tag="big")
                for n in range(2):
                    for i in range(4):
                        nc.tensor.matmul(
                            out=qk_ps[:, 512 * n : 512 * n + 512],
                            lhsT=w1t_sb[:, i, 128 * j : 128 * j + 128],
                            rhs=xn_sb[:, i, 512 * n : 512 * n + 512],
                            start=(i == 0),
                            stop=(i == 3),
                        )
                dst = q_sb[:, j, :] if j < 4 else k_sb[:, j - 4, :]
                nc.vector.tensor_scalar(
                    out=dst,
                    in0=qk_ps,
                    scalar1=b1r_sb[:, j : j + 1],
                    scalar2=SCALE,
                    op0=Alu.add,
                    op1=Alu.mult,
                )

            # v^T: stationary = xn chunk, moving = W1v^T
            for st in range(8):
                vt_ps = ps.tile([128, T], f32, tag="big")
                for i in range(4):
                    nc.tensor.matmul(
                        out=vt_ps[:, 0:512],
                        lhsT=xn_sb[:, i, 128 * st : 128 * st + 128],
                        rhs=w1t_sb[:, i, 2 * C : 3 * C],
                        start=(i == 0),
                        stop=(i == 3),
                    )
                nc.vector.scalar_tensor_tensor(
                    out=vt_sb[:, st, :, 0:64],
                    in0=vt_ps[:, 0:512].rearrange("p (h c) -> p h c", c=64),
                    scalar=1.0,
                    in1=b1v_bc.rearrange("p (h c) -> p h c", c=64),
                    op0=Alu.mult,
                    op1=Alu.add,
                )

            if debug_taps:
                pass  # dbg_xn tap disabled in bf16 build
                pass
                pass
                pass

            # second copy of x, head-aligned: partition = channel within head.
            # Loaded late so it doesn't compete with x/w1t for DMA at startup.
            x_hd = cst.tile([64, 8, T], f32)
            nc.gpsimd.dma_start(out=x_hd, in_=x_d.ap().rearrange("(h p) t -> p h t", p=64))

            # ------------- attention: paired heads, row-tiled scores -------------
            # Heads 2j/2j+1 share each [128,1024] score tile: cols 0:512 carry
            # head A's (st,n) chunk, 512:1024 head B's. The two K=64 score
            # matmuls get explicit tile_position (0,0)/(64,0) so they run
            # concurrently in disjoint PE row groups. Pass A fills 16 wt tiles
            # per pair; pass B runs the 32 AV matmuls dependency-free, which
            # also overlaps the next pair's pass A.
            for j in range(HEADS // 2):
                hA, hB = 2 * j, 2 * j + 1
                wts = []
                for st in range(8):
                    for n in range(2):
                        st_ps = ps.tile(
                            [128, T], f32, tag="big", name=f"st_{j}_{st}_{n}"
                        )
                        for hi, h in enumerate((hA, hB)):
                            hp = (h % 2) * 64
                            nc.tensor.matmul(
                                out=st_ps[:, 512 * hi : 512 * hi + 512],
                                lhsT=k_sb[hp : hp + 64, j, 128 * st : 128 * st + 128],
                                rhs=q_sb[hp : hp + 64, j, 512 * n : 512 * n + 512],
                                start=True,
                                stop=True,
                                tile_position=(hp, 0),
                            )
                        wt = wtp.tile(
                            [128, T], bf16, tag="wt", bufs=20, name=f"wt_{j}_{st}_{n}"
                        )
                        nc.scalar.activation(
                            out=wt, in_=st_ps, func=Act.Exp, bias=0.0, scale=1.0
                        )
                        wts.append(wt)
                av = {
                    hA: ps.tile([128, T], f32, tag="av", name=f"av_{hA}"),
                    hB: ps.tile([128, T], f32, tag="av", name=f"av_{hB}"),
                }
                for st in range(8):
                    for n in range(2):
                        wt = wts[2 * st + n]
                        for hi, h in enumerate((hA, hB)):
                            nc.tensor.matmul(
                                out=av[h][0:65, 512 * n : 512 * n + 512],
                                lhsT=vt_sb[:, st, h, 0:65],
                                rhs=wt[:, 512 * hi : 512 * hi + 512],
                                start=(st == 0),
                                stop=(st == 7),
                            )
                for h in (hA, hB):
                    av_ps = av[h]
                    if debug_taps and h == 0:
                        av_cp = outp.tile([128, T], f32, tag="avcp", bufs=1)
                        nc.vector.tensor_copy(out=av_cp[0:65, :], in_=av_ps[0:65, :])
                        nc.sync.dma_start(out=dbg_av[:, :], in_=av_cp)
                    # Evacuate PSUM immediately; reciprocal off-PSUM via DMA
                    # reshape -> all-lane DVE recip -> row -> row-broadcast.
                    o_st = outp.tile([64, T], f32, tag="o", name=f"o_{h}")
                    nc.vector.tensor_copy(out=o_st, in_=av_ps[0:64, :])
                    rrow = wtp.tile([128, T], f32, tag="rrow", bufs=2, name=f"rrow_{h}")
                    nc.vector.tensor_copy(out=rrow[64:65, :], in_=av_ps[64:65, :])
                    rsp = wtp.tile([128, 8], f32, tag="rsp", bufs=2, name=f"rsp_{h}")
                    nc.sync.dma_start(out=rsp, in_=rrow[64:65, :])
                    rsp2 = wtp.tile([128, 8], f32, tag="rsp2", bufs=2, name=f"rsp2_{h}")
                    nc.vector.reciprocal(out=rsp2, in_=rsp)
                    nc.sync.dma_start(out=rrow[0:1, :], in_=rsp2)
                    rbc = wtp.tile([64, T], f32, tag="rb", bufs=2, name=f"rbc_{h}")
                    srcap = rrow[0:1, :]
                    nc.gpsimd.dma_start(
                        out=rbc,
                        in_=bass.AP(
                            tensor=srcap.tensor,
                            offset=srcap.offset,
                            ap=[srcap.ap[0], [0, 64], srcap.ap[1]],
                        ),
                    )
                    if debug_taps and h == 0:
                        nc.sync.dma_start(out=dbg_r[0:64, :], in_=rbc)
                    nc.vector.tensor_mul(o_st, o_st, rbc)
                    nc.vector.tensor_add(o_st, o_st, x_hd[:, h, :])
                    nc.gpsimd.dma_start(out=out_d[64 * h : 64 * h + 64, :], in_=o_st)

    nc.finalize()
    return nc


def _make_in_maps(inputs):
    x = np.ascontiguousarray(np.asarray(inputs["x"], dtype=np.float32))
    gnw = np.asarray(inputs["gn_weight"], dtype=np.float32)
    gnb = np.asarray(inputs["gn_bias"], dtype=np.float32)
    w1 = np.asarray(inputs["w1"], dtype=np.float32)
    b1 = np.asarray(inputs["b1"], dtype=np.float32)

    import ml_dtypes

    B = x.shape[0]
    w1t = np.ascontiguousarray(w1[:, :, 0].T).astype(ml_dtypes.bfloat16)  # [C, 3C]
    b1r = np.ascontiguousarray(b1.reshape(12, 128).T)              # [128, 12]
    b1v = np.ascontiguousarray(b1[2 * C : 3 * C].reshape(1, C))    # [1, C]
    gnw_r = np.ascontiguousarray(gnw.reshape(4, 128).T)            # [128, 4]
    gnb_r = np.ascontiguousarray(gnb.reshape(4, 128).T)            # [128, 4]

    ind16 = np.zeros((128, 8), np.float32)
    indT = np.zeros((8, 128), np.float32)
    for g in range(8):
        ind16[16 * g : 16 * g + 16, g] = 1.0 / 16.0
        indT[g, 16 * g : 16 * g + 16] = 1.0

    in_maps = []
    for b in range(B):
        in_maps.append(
            {
                "x": np.ascontiguousarray(x[b].reshape(C, T)),
                "w1t": w1t,
                "b1r": b1r,
                "b1v": b1v,
                "gnw": gnw_r,
                "gnb": gnb_r,
                "ind16": ind16,
                "indT": indT,
            }
        )
    return in_maps


def _gather(results, x_shape):
    B, Cc, H, W = x_shape
    out = np.empty((B, Cc, H, W), dtype=np.float32)
    for b in range(B):
        out[b] = results[b]["out"].reshape(Cc, H, W)
    return out


def kernel(**inputs):
    from concourse.bass_utils import run_bass_kernel_spmd

    nc = _build_nc()
    in_maps = _make_in_maps(inputs)
    res = run_bass_kernel_spmd(nc, in_maps, core_ids=list(range(N_CORES)))
    return _gather(res.results, np.asarray(inputs["x"]).shape)



# revision 8
# speedup vs baseline: 1.0332x; 1.0332x over previous
"""Trainium2 Bass kernel for nn_Attention_64235530879146.

Reference computation (per batch element, C=512, T=H*W=1024, 32 groups,
8 heads of ch=64):
    xn = GroupNorm(x) * gn_weight + gn_bias          # [C, T]
    qkv = W1 @ xn + b1                               # [3C, T]
    per head: St[s,t] = (k*sc)^T (q*sc),  sc = ch**-0.25
              Wt = exp(St)   (no max subtraction; |S| < 8 for N(0,1) inputs,
                              far inside fp32 exp range)
              a[c,t] = sum_s v[c,s] Wt[s,t] / r[t],  r[t] = sum_s Wt[s,t]
    out = a + x
Sharding: pure data-parallel over batch - 8 batch elements on 8 NeuronCores,
no collectives.

The ScalarE exp stream (64 ACTIVATEs of [128,1024] = ~75 us with sem
overhead) is the hard floor for this problem. The whole schedule is built
to (a) start that stream as early as possible, (b) never starve it, and
(c) keep the PE dense enough that HAM never re-throttles the clock:

  - GroupNorm: bn_stats/bn_aggr per channel; group reduce + broadcast via
    tiny PE matmuls with indicator matrices. rstd = exp(-0.5*ln(var+eps))
    so the whole kernel uses one ACT table set (natural_log_exp).
  - x is loaded over two DMA queues in parallel; q0/k0 projection chunks
    run first so pair-0 scores start at ~12 us.
  - The remaining QKV projection chunks are interleaved one per attention
    step into the pair-0/1/2 score streams (so the PE never idles and the
    exp stream never waits long on a projection burst).
  - scores: per (pair, head, s-chunk) one [128,1024] f32 PSUM tile, 2
    N=512 matmuls; the two heads of a pair use tile_position (0,0)/(64,0)
    to run in disjoint PE row groups. ACT Exp PSUM->SBUF (bf16) FD=1024.
  - AV (lhsT = vT_aug[128,65], ones column emits the softmax denominator r
    in row 64) is interleaved into the NEXT pair's score stream (one-pair
    lag, compressed 2-per-step from pair 2 on) so it fills PE slack
    between score matmuls instead of forming serial phases.
  - PSUM (8 banks): scores+projection+GN tiles share a 2-slot x 2-bank
    pool; AV accumulators get 2 slots x 2 banks.
  - Epilogue per head, fully pipelined across heads and overlapped with
    the next pair's exp stream: copy [65,T] PSUM->SBUF (frees the AV
    slot), DMA-reshape r to [128,8] (vector queue), all-lane DVE
    reciprocal, DMA back to a row, DMA row-broadcast to 64 partitions
    (gpsimd queue), in-place o*=1/r and o+=x on DVE, store on the Sync
    queue.

Matmul inputs are bf16 (fp32 PSUM accumulate): measured end-to-end relative
error vs an fp64 reference is ~3.5e-4. Weights are transposed/reformatted on
the host in _make_in_maps (pure layout prep, no arithmetic beyond a bf16
cast).
"""
import numpy as np

GROUPS = 32
HEADS = 8
EPS = 1e-5
C = 512
T = 1024
CH = C // HEADS            # 64
SCALE = float(CH) ** -0.25
N_CORES = 8


def _build_nc():
    import concourse.bass as bass
    import concourse.mybir as mybir
    import concourse.tile as tile
    from concourse import bacc

    f32 = mybir.dt.float32
    bf16 = mybir.dt.bfloat16
    Alu = mybir.AluOpType
    Act = mybir.ActivationFunctionType

    nc = bacc.Bacc("TRN2", target_bir_lowering=False, debug=False)

    x_d = nc.declare_dram_parameter("x", [C, T], f32, isOutput=False)
    w1t_d = nc.declare_dram_parameter("w1t", [C, 3 * C], bf16, isOutput=False)
    b1r_d = nc.declare_dram_parameter("b1r", [128, 12], f32, isOutput=False)
    b1v_d = nc.declare_dram_parameter("b1v", [1, C], f32, isOutput=False)
    gnw_d = nc.declare_dram_parameter("gnw", [128, 4], f32, isOutput=False)
    gnb_d = nc.declare_dram_parameter("gnb", [128, 4], f32, isOutput=False)
    ind16_d = nc.declare_dram_parameter("ind16", [128, 8], f32, isOutput=False)
    indT_d = nc.declare_dram_parameter("indT", [8, 128], f32, isOutput=False)
    out_d = nc.declare_dram_parameter("out", [C, T], f32, isOutput=True)

    with tile.TileContext(nc) as tc:
        with (
            tc.tile_pool(name="const", bufs=1) as cst,
            tc.tile_pool(name="work", bufs=2) as work,
            tc.tile_pool(name="wtp", bufs=4) as wtp,
            tc.tile_pool(name="ps", bufs=2, space="PSUM") as ps,
        ):
            # ---------------- loads ----------------
            # x c-tiles split over the Sync and Vector HWDGE queues so the
            # last tile lands in ~3 us; w1t q/k columns first on the ACT
            # queue; the head-aligned residual copy of x trails on the
            # Vector queue (first needed ~40 us in); small constants on
            # GpSimd.
            xv = x_d.ap().rearrange("(i p) t -> i p t", p=128)
            x_sb = cst.tile([128, 4, T], f32)
            for i in range(2):
                nc.sync.dma_start(out=x_sb[:, i, :], in_=xv[i])
            for i in range(2, 4):
                nc.scalar.dma_start(out=x_sb[:, i, :], in_=xv[i])
            w1t_sb = cst.tile([128, 4, 3 * C], bf16)
            w1tv = w1t_d.ap().rearrange("(i p) o -> p i o", p=128)
            nc.scalar.dma_start(out=w1t_sb[:, :, : 2 * C], in_=w1tv[:, :, : 2 * C])
            nc.scalar.dma_start(out=w1t_sb[:, :, 2 * C :], in_=w1tv[:, :, 2 * C :])
            b1r_sb = cst.tile([128, 12], f32)
            nc.gpsimd.dma_start(out=b1r_sb, in_=b1r_d[:, :])
            gnw_sb = cst.tile([128, 4], f32)
            nc.gpsimd.dma_start(out=gnw_sb, in_=gnw_d[:, :])
            gnb_sb = cst.tile([128, 4], f32)
            nc.gpsimd.dma_start(out=gnb_sb, in_=gnb_d[:, :])
            b1v_bc = cst.tile([128, C], f32)
            nc.gpsimd.dma_start(out=b1v_bc, in_=b1v_d.ap().to_broadcast((128, C)))

            # group indicator constants (from host): ind16[c,g]=1/16, indT[g,c]=1
            ind16 = cst.tile([128, 8], f32)
            nc.gpsimd.dma_start(out=ind16, in_=ind16_d[:, :])
            indT = cst.tile([8, 128], f32)
            nc.gpsimd.dma_start(out=indT, in_=indT_d[:, :])
            eps8 = cst.tile([8, 1], f32)
            nc.vector.memset(eps8, EPS)

            # second copy of x, head-aligned: partition = channel within head.
            x_hd = cst.tile([64, 8, T], f32)
            nc.sync.dma_start(out=x_hd, in_=x_d.ap().rearrange("(h p) t -> p h t", p=64))

            # ---------------- GroupNorm stats ----------------
            # per-channel bn stats -> [mean, var, mean^2] per 128-channel tile
            rhs3 = cst.tile([128, 4, 3], f32)
            for i in range(4):
                st6 = work.tile([128, 2, 6], f32, tag="st6")
                nc.vector.bn_stats(out=st6[:, 0, :], in_=x_sb[:, i, 0:512])
                nc.vector.bn_stats(out=st6[:, 1, :], in_=x_sb[:, i, 512:1024])
                mv = work.tile([128, 2], f32, tag="mv")
                nc.vector.bn_aggr(out=mv, in_=st6)
                nc.vector.tensor_copy(out=rhs3[:, i, 0:2], in_=mv)
                nc.vector.tensor_mul(rhs3[:, i, 2:3], mv[:, 0:1], mv[:, 0:1])

            # reduce 16-channel groups via PE: [8 groups, (mu, Evar, Emu2)] per tile
            stats_ps = ps.tile([8, 12], f32, tag="big")
            for i in range(4):
                nc.tensor.matmul(
                    out=stats_ps[:, 3 * i : 3 * i + 3],
                    lhsT=ind16,
                    rhs=rhs3[:, i, :],
                    start=True,
                    stop=True,
                )
            sg = cst.tile([8, 12], f32)
            nc.vector.tensor_copy(out=sg, in_=stats_ps)
            # musig[:, 0, i] = mu_g ; musig[:, 1, i] = rstd_g
            musig = cst.tile([8, 2, 4], f32)
            mu_v = sg.rearrange("p (i three) -> p i three", three=3)
            nc.vector.tensor_copy(out=musig[:, 0, :], in_=mu_v[:, :, 0])
            var_g = cst.tile([8, 4], f32)
            nc.vector.tensor_add(var_g, mu_v[:, :, 1], mu_v[:, :, 2])
            mu2 = cst.tile([8, 4], f32)
            nc.vector.tensor_mul(mu2, mu_v[:, :, 0], mu_v[:, :, 0])
            nc.vector.tensor_sub(var_g, var_g, mu2)
            # rstd = 1/sqrt(var+eps) = exp(-0.5*ln(var+eps)); Ln and Exp share
            # one ACT table set, so no sqrt-set reload mid-kernel.
            lnv = cst.tile([8, 4], f32)
            nc.scalar.activation(out=lnv, in_=var_g, func=Act.Ln, bias=eps8, scale=1.0)
            nc.scalar.activation(
                out=musig[:, 1, :], in_=lnv, func=Act.Exp, bias=0.0, scale=-0.5
            )

            # broadcast (mu, rstd) back to channels; fold gn affine:
            # a_c = gnw * rstd ; b_c = gnb - mu * a_c ; xn = x*a_c + b_c
            xn_sb = cst.tile([128, 4, T], bf16)
            af = cst.tile([128, 4, 2], f32)
            for i in range(4):
                musig_ps = ps.tile([128, 2], f32, tag="big")
                nc.tensor.matmul(
                    out=musig_ps, lhsT=indT, rhs=musig[:, :, i], start=True, stop=True
                )
                nc.vector.tensor_mul(af[:, i, 0:1], gnw_sb[:, i : i + 1], musig_ps[:, 1:2])
                tmp = work.tile([128, 1], f32, tag="tmp1")
                nc.vector.tensor_mul(tmp, musig_ps[:, 0:1], af[:, i, 0:1])
                nc.vector.tensor_sub(af[:, i, 1:2], gnb_sb[:, i : i + 1], tmp)
                nc.vector.tensor_scalar(
                    out=xn_sb[:, i, :],
                    in0=x_sb[:, i, :],
                    scalar1=af[:, i, 0:1],
                    scalar2=af[:, i, 1:2],
                    op0=Alu.mult,
                    op1=Alu.add,
                )

            # ---------------- QKV building blocks ----------------
            q_sb = cst.tile([128, 4, T], bf16)
            k_sb = cst.tile([128, 4, T], bf16)
            # vT_aug layout per (s_tile, head): v^T at cols 0:64, ones at col 64.
            # The ones column makes the AV matmul also emit the softmax denom r.
            vt_sb = cst.tile([128, 8, 8, 65], bf16)
            nc.vector.tensor_copy(
                out=vt_sb[:, :, :, 64:65],
                in_=nc.const_aps.tensor(1.0, (128, 8, 8, 1), bf16),
            )

            def emit_qk_half(j, n):
                # o-chunk j (j<4: q rows 128j, else k rows 128(j-4)), t-half n
                qk_ps = ps.tile([128, 512], f32, tag="big", name=f"qk_{j}_{n}")
                for i in range(4):
                    nc.tensor.matmul(
                        out=qk_ps,
                        lhsT=w1t_sb[:, i, 128 * j : 128 * j + 128],
                        rhs=xn_sb[:, i, 512 * n : 512 * n + 512],
                        start=(i == 0),
                        stop=(i == 3),
                    )
                dst = q_sb if j < 4 else k_sb
                nc.vector.tensor_scalar(
                    out=dst[:, j % 4, 512 * n : 512 * n + 512],
                    in0=qk_ps,
                    scalar1=b1r_sb[:, j : j + 1],
                    scalar2=SCALE,
                    op0=Alu.add,
                    op1=Alu.mult,
                )

            def emit_v(st):
                # v^T s-chunk: stationary = xn chunk, moving = W1v^T
                vt_ps = ps.tile([128, 512], f32, tag="big", name=f"vt_{st}")
                for i in range(4):
                    nc.tensor.matmul(
                        out=vt_ps,
                        lhsT=xn_sb[:, i, 128 * st : 128 * st + 128],
                        rhs=w1t_sb[:, i, 2 * C : 3 * C],
                        start=(i == 0),
                        stop=(i == 3),
                    )
                nc.vector.scalar_tensor_tensor(
                    out=vt_sb[:, st, :, 0:64],
                    in0=vt_ps.rearrange("p (h c) -> p h c", c=64),
                    scalar=1.0,
                    in1=b1v_bc.rearrange("p (h c) -> p h c", c=64),
                    op0=Alu.mult,
                    op1=Alu.add,
                )

            # ---------------- attention building blocks ----------------
            wts = {}

            def emit_score_exp(p, st, hi):
                h = 2 * p + hi
                hp = 64 * hi
                st_ps = ps.tile([128, T], f32, tag="big", name=f"st_{p}_{st}_{hi}")
                for n in range(2):
                    nc.tensor.matmul(
                        out=st_ps[:, 512 * n : 512 * n + 512],
                        lhsT=k_sb[hp : hp + 64, p, 128 * st : 128 * st + 128],
                        rhs=q_sb[hp : hp + 64, p, 512 * n : 512 * n + 512],
                        start=True,
                        stop=True,
                        tile_position=(hp, 0),
                    )
                wt = wtp.tile([128, T], bf16, tag="wt", bufs=24, name=f"wt_{p}_{st}_{hi}")
                nc.scalar.activation(out=wt, in_=st_ps, func=Act.Exp, bias=0.0, scale=1.0)
                wts[(p, st, hi)] = wt

            av_tiles = {}

            def av_of(p):
                if p not in av_tiles:
                    av_tiles[p] = {
                        hi: ps.tile([128, T], f32, tag="av", name=f"av_{p}_{hi}")
                        for hi in range(2)
                    }
                return av_tiles[p]

            def emit_av(p, st):
                av = av_of(p)
                for hi in range(2):
                    h = 2 * p + hi
                    wt = wts.pop((p, st, hi))
                    for n in range(2):
                        nc.tensor.matmul(
                            out=av[hi][0:65, 512 * n : 512 * n + 512],
                            lhsT=vt_sb[:, st, h, 0:65],
                            rhs=wt[:, 512 * n : 512 * n + 512],
                            start=(st == 0),
                            stop=(st == 7),
                        )

            def emit_epilogue(p):
                av = av_tiles.pop(p)
                for hi in range(2):
                    h = 2 * p + hi
                    # copy a and r out of PSUM in one op (frees the AV slot),
                    # then the reciprocal chain: DMA-reshape r to [128,8]
                    # (all-lane DVE recip), back to a row, row-broadcast to
                    # the 64 channel partitions. Runs concurrently with the
                    # next pair's exp stream.
                    o65 = wtp.tile([65, T], f32, tag="o65", bufs=3, name=f"o_{h}")
                    nc.vector.tensor_copy(out=o65, in_=av[hi][0:65, :])
                    rsp = wtp.tile([128, 8], f32, tag="rsp", bufs=3, name=f"rsp_{h}")
                    nc.sync.dma_start(out=rsp, in_=o65[64:65, :])
                    rsp2 = wtp.tile([128, 8], f32, tag="rsp2", bufs=3, name=f"rsp2_{h}")
                    nc.vector.reciprocal(out=rsp2, in_=rsp)
                    rrow = wtp.tile([1, T], f32, tag="rrow", bufs=3, name=f"rrow_{h}")
                    nc.sync.dma_start(out=rrow, in_=rsp2)
                    rbc = wtp.tile([64, T], f32, tag="rb", bufs=3, name=f"rbc_{h}")
                    srcap = rrow[0:1, :]
                    nc.gpsimd.dma_start(
                        out=rbc,
                        in_=bass.AP(
                            tensor=srcap.tensor,
                            offset=srcap.offset,
                            ap=[srcap.ap[0], [0, 64], srcap.ap[1]],
                        ),
                    )
                    nc.vector.tensor_mul(o65[0:64, :], o65[0:64, :], rbc)
                    nc.vector.tensor_add(o65[0:64, :], o65[0:64, :], x_hd[:, h, :])
                    nc.sync.dma_start(out=out_d[64 * h : 64 * h + 64, :], in_=o65[0:64, :])

            # ---------------- the interleaved schedule ----------------
            # q0/k0 first so pair-0 scores can start immediately.
            emit_qk_half(0, 0)
            emit_qk_half(0, 1)
            emit_qk_half(4, 0)
            emit_qk_half(4, 1)

            # one projection unit per attention step, placed so every consumer
            # deadline is met: qk_j before pair j%4's scores, v_st before the
            # AV that reads it.
            units = {
                0: [lambda: emit_qk_half(1, 0), lambda: emit_qk_half(1, 1),
                    lambda: emit_qk_half(5, 0), lambda: emit_qk_half(5, 1),
                    lambda: emit_v(0), lambda: emit_v(1),
                    lambda: emit_v(2), lambda: emit_v(3)],
                1: [lambda: emit_v(4), lambda: emit_v(5),
                    lambda: emit_v(6), lambda: emit_v(7),
                    lambda: emit_qk_half(2, 0), lambda: emit_qk_half(2, 1),
                    lambda: emit_qk_half(6, 0), lambda: emit_qk_half(6, 1)],
                2: [lambda: emit_qk_half(3, 0), lambda: emit_qk_half(3, 1),
                    lambda: emit_qk_half(7, 0), lambda: emit_qk_half(7, 1),
                    None, None, None, None],
                3: [None] * 8,
            }
            # AV placement: pair-0's AV spreads over pair-1's steps (one per
            # step, after the producing exp AND the v chunk it needs); later
            # pairs compress 2-per-step so pair-3's own AV can start mid-pass
            # and the tail stays short.
            av_sched = {
                0: {},
                1: {s: [(0, s)] for s in range(8)},
                2: {s: [(1, 2 * s), (1, 2 * s + 1)] for s in range(4)},
                3: {
                    **{s: [(2, 2 * s), (2, 2 * s + 1)] for s in range(4)},
                    5: [(3, 0), (3, 1)],
                    6: [(3, 2), (3, 3)],
                    7: [(3, 4), (3, 5)],
                },
            }

            for p in range(4):
                for s in range(8):
                    emit_score_exp(p, s, 0)
                    emit_score_exp(p, s, 1)
                    u = units[p][s]
                    if u is not None:
                        u()
                    for (pp, st) in av_sched[p].get(s, []):
                        emit_av(pp, st)
                    if p == 2 and s == 3:
                        emit_epilogue(1)
                    if p == 3 and s == 3:
                        emit_epilogue(2)
                if p == 1:
                    emit_epilogue(0)
            emit_av(3, 6)
            emit_av(3, 7)
            emit_epilogue(3)

    nc.finalize()
    return nc


def _make_in_maps(inputs):
    x = np.ascontiguousarray(np.asarray(inputs["x"], dtype=np.float32))
    gnw = np.asarray(inputs["gn_weight"], dtype=np.float32)
    gnb = np.asarray(inputs["gn_bias"], dtype=np.float32)
    w1 = np.asarray(inputs["w1"], dtype=np.float32)
    b1 = np.asarray(inputs["b1"], dtype=np.float32)

    import ml_dtypes

    B = x.shape[0]
    w1t = np.ascontiguousarray(w1[:, :, 0].T).astype(ml_dtypes.bfloat16)  # [C, 3C]
    b1r = np.ascontiguousarray(b1.reshape(12, 128).T)              # [128, 12]
    b1v = np.ascontiguousarray(b1[2 * C : 3 * C].reshape(1, C))    # [1, C]
    gnw_r = np.ascontiguousarray(gnw.reshape(4, 128).T)            # [128, 4]
    gnb_r = np.ascontiguousarray(gnb.reshape(4, 128).T)            # [128, 4]

    ind16 = np.zeros((128, 8), np.float32)
    indT = np.zeros((8, 128), np.float32)
    for g in range(8):
        ind16[16 * g : 16 * g + 16, g] = 1.0 / 16.0
        indT[g, 16 * g : 16 * g + 16] = 1.0

    in_maps = []
    for b in range(B):
        in_maps.append(
            {
                "x": np.ascontiguousarray(x[b].reshape(C, T)),
                "w1t": w1t,
                "b1r": b1r,
                "b1v": b1v,
                "gnw": gnw_r,
                "gnb": gnb_r,
                "ind16": ind16,
                "indT": indT,
            }
        )
    return in_maps


def _gather(results, x_shape):
    B, Cc, H, W = x_shape
    out = np.empty((B, Cc, H, W), dtype=np.float32)
    for b in range(B):
        out[b] = results[b]["out"].reshape(Cc, H, W)
    return out


def kernel(**inputs):
    from concourse.bass_utils import run_bass_kernel_spmd

    nc = _build_nc()
    in_maps = _make_in_maps(inputs)
    res = run_bass_kernel_spmd(nc, in_maps, core_ids=list(range(N_CORES)))
    return _gather(res.results, np.asarray(inputs["x"]).shape)


# revision 9
# speedup vs baseline: 1.2822x; 1.2410x over previous
"""Trainium2 Bass kernel for nn_Attention_64235530879146.

Reference computation (per batch element, C=512, T=H*W=1024, 32 groups,
8 heads of ch=64):
    xn = GroupNorm(x) * gn_weight + gn_bias          # [C, T]
    qkv = W1 @ xn + b1                               # [3C, T]
    per head: St[s,t] = (k*sc)^T (q*sc),  sc = ch**-0.25
              Wt = exp(St)   (no max subtraction; |S| < 8 for N(0,1) inputs,
                              far inside fp32 exp range)
              a[c,t] = sum_s v[c,s] Wt[s,t] / r[t],  r[t] = sum_s Wt[s,t]
    out = a + x
Sharding: pure data-parallel over batch - 8 batch elements on 8 NeuronCores,
no collectives.

The ScalarE exp stream (64 ACTIVATEs of [128,1024] = ~75 us with sem
overhead) is the hard floor for this problem. The schedule starts that
stream as early as possible and never lets it starve:

  - DMA: one queue carries ~100 GB/s, so x is split over the Sync and ACT
    HWDGE queues, and w1t is chunked in *consumption* order (q0/q1+k4/k5
    columns right after x, v and late head chunks after) so the first
    score matmul is gated by GroupNorm, not weights.
  - GroupNorm runs as four independent per-c-tile pipelines (the 16-channel
    groups never cross a 128-channel tile): bn_stats -> tiny PE group
    reduce -> Sqrt (ACT) -> DVE reciprocal -> PE broadcast -> fused affine.
    The affine xn = a_c*x + b_c runs on ACT (Identity with per-partition
    scale AND bias APs), which keeps the DVE chain off the critical path.
    Tiles are processed in DMA-arrival order (0,2,1,3 - sync/scalar queues
    interleave).
  - scores: per (pair, head, s-chunk) one [128,1024] f32 PSUM tile, 2
    N=512 matmuls; the two heads of a pair use tile_position (0,0)/(64,0)
    to run in disjoint PE row groups. ACT Exp PSUM->SBUF (bf16) FD=1024.
  - The remaining QKV projection chunks are interleaved one per attention
    step; AV (lhsT = vT_aug[128,65], ones column emits the softmax
    denominator r) runs with a one-pair lag inside the next pair's score
    stream, compressed 2-per-step from pair 2 so pair 3's own AV starts
    mid-pass and the tail stays short.
  - PSUM (8 banks): scores+projections+GN share a 2-slot x 2-bank pool;
    AV accumulators get 2 slots x 2 banks.
  - Epilogue per head is cut into three stages emitted several attention
    steps apart, so no DVE/queue head-of-line blocking: (a) copy [65,T]
    PSUM->SBUF (frees the AV slot) + DMA-reshape r to [128,8]; (b)
    all-lane DVE reciprocal + DMA back to a row + GpSimd
    partition_broadcast to the 64 channel lanes (a Q7 compute op - the
    DMA row-broadcast runs at ~25 GB/s and stalled the whole pipeline);
    (c) in-place o*=1/r, o+=x, store on the Sync queue.

Matmul inputs are bf16 (fp32 PSUM accumulate): measured end-to-end relative
error vs an fp64 reference is ~3.5e-4. Weights are transposed/reformatted on
the host in _make_in_maps (pure layout prep, no arithmetic beyond a bf16
cast).
"""
import numpy as np

GROUPS = 32
HEADS = 8
EPS = 1e-5
C = 512
T = 1024
CH = C // HEADS            # 64
SCALE = float(CH) ** -0.25
N_CORES = 8

# c-tile processing order = DMA arrival order (x0,x1 on sync; x2,x3 on scalar)
ARR = (0, 2, 1, 3)


def _build_nc():
    import concourse.bass as bass
    import concourse.mybir as mybir
    import concourse.tile as tile
    from concourse import bacc
    from concourse import library_config

    f32 = mybir.dt.float32
    bf16 = mybir.dt.bfloat16
    Alu = mybir.AluOpType
    Act = mybir.ActivationFunctionType

    nc = bacc.Bacc("TRN2", target_bir_lowering=False, debug=False)

    x_d = nc.declare_dram_parameter("x", [C, T], f32, isOutput=False)
    w1t_d = nc.declare_dram_parameter("w1t", [C, 3 * C], bf16, isOutput=False)
    b1r_d = nc.declare_dram_parameter("b1r", [128, 12], f32, isOutput=False)
    b1v_d = nc.declare_dram_parameter("b1v", [1, C], f32, isOutput=False)
    gnw_d = nc.declare_dram_parameter("gnw", [128, 4], f32, isOutput=False)
    gnb_d = nc.declare_dram_parameter("gnb", [128, 4], f32, isOutput=False)
    ind16_d = nc.declare_dram_parameter("ind16", [128, 8], f32, isOutput=False)
    indT_d = nc.declare_dram_parameter("indT", [8, 128], f32, isOutput=False)
    out_d = nc.declare_dram_parameter("out", [C, T], f32, isOutput=True)

    with tile.TileContext(nc) as tc:
        with (
            tc.tile_pool(name="const", bufs=1) as cst,
            tc.tile_pool(name="work", bufs=2) as work,
            tc.tile_pool(name="wtp", bufs=4) as wtp,
            tc.tile_pool(name="ps", bufs=2, space="PSUM") as ps,
        ):
            # partition_broadcast lives in the `attn` GpSimd library.
            nc.gpsimd.load_library(library_config.attn)

            # ---------------- loads ----------------
            xv = x_d.ap().rearrange("(i p) t -> i p t", p=128)
            x_sb = cst.tile([128, 4, T], f32)
            nc.sync.dma_start(out=x_sb[:, 0, :], in_=xv[0])
            nc.sync.dma_start(out=x_sb[:, 1, :], in_=xv[1])
            nc.scalar.dma_start(out=x_sb[:, 2, :], in_=xv[2])
            nc.scalar.dma_start(out=x_sb[:, 3, :], in_=xv[3])
            # w1t in consumption order: q0/q1 cols, k4/k5 cols, v cols,
            # q2/q3 cols, k6/k7 cols.
            w1t_sb = cst.tile([128, 4, 3 * C], bf16)
            w1tv = w1t_d.ap().rearrange("(i p) o -> p i o", p=128)
            for lo, hi in ((0, 256), (512, 768), (1024, 1536), (256, 512), (768, 1024)):
                nc.scalar.dma_start(out=w1t_sb[:, :, lo:hi], in_=w1tv[:, :, lo:hi])
            # head-aligned residual copy of x (needed only from ~45 us on)
            x_hd = cst.tile([64, 8, T], f32)
            nc.sync.dma_start(out=x_hd, in_=x_d.ap().rearrange("(h p) t -> p h t", p=64))

            b1r_sb = cst.tile([128, 12], f32)
            nc.gpsimd.dma_start(out=b1r_sb, in_=b1r_d[:, :])
            gnw_sb = cst.tile([128, 4], f32)
            nc.gpsimd.dma_start(out=gnw_sb, in_=gnw_d[:, :])
            gnb_sb = cst.tile([128, 4], f32)
            nc.gpsimd.dma_start(out=gnb_sb, in_=gnb_d[:, :])
            b1v_bc = cst.tile([128, C], f32)
            nc.gpsimd.dma_start(out=b1v_bc, in_=b1v_d.ap().to_broadcast((128, C)))
            ind16 = cst.tile([128, 8], f32)
            nc.gpsimd.dma_start(out=ind16, in_=ind16_d[:, :])
            indT = cst.tile([8, 128], f32)
            nc.gpsimd.dma_start(out=indT, in_=indT_d[:, :])
            eps8 = cst.tile([8, 1], f32)
            nc.vector.memset(eps8, EPS)

            # ---------------- GroupNorm: four per-tile pipelines ----------------
            # Groups are 16 channels, fully inside one 128-channel tile, so
            # each tile computes stats -> rstd -> affine independently and
            # feeds the QKV accumulation as soon as it's done.
            xn_sb = cst.tile([128, 4, T], bf16)
            af = cst.tile([128, 4, 2], f32)

            def gn_stats(i):
                st6 = work.tile([128, 2, 6], f32, tag="st6")
                nc.vector.bn_stats(out=st6[:, 0, :], in_=x_sb[:, i, 0:512])
                nc.vector.bn_stats(out=st6[:, 1, :], in_=x_sb[:, i, 512:1024])
                mv = work.tile([128, 2], f32, tag="mv")
                nc.vector.bn_aggr(out=mv, in_=st6)
                rhs3 = work.tile([128, 3], f32, tag="rhs3")
                nc.vector.tensor_copy(out=rhs3[:, 0:2], in_=mv)
                nc.vector.tensor_mul(rhs3[:, 2:3], mv[:, 0:1], mv[:, 0:1])
                return rhs3

            def gn_finish(i, rhs3):
                # group reduce: [8, (mu, Evar, Emu2)] for this tile's 8 groups
                sps = ps.tile([8, 3], f32, tag="big", name=f"gn_{i}")
                nc.tensor.matmul(out=sps, lhsT=ind16, rhs=rhs3, start=True, stop=True)
                sg = work.tile([8, 3], f32, tag="sg")
                nc.vector.tensor_copy(out=sg, in_=sps)
                musig = work.tile([8, 2], f32, tag="musig")
                nc.vector.tensor_copy(out=musig[:, 0:1], in_=sg[:, 0:1])
                var_g = work.tile([8, 1], f32, tag="varg")
                nc.vector.tensor_add(var_g, sg[:, 1:2], sg[:, 2:3])
                mu2 = work.tile([8, 1], f32, tag="mu2")
                nc.vector.tensor_mul(mu2, sg[:, 0:1], sg[:, 0:1])
                nc.vector.tensor_sub(var_g, var_g, mu2)
                sdv = work.tile([8, 1], f32, tag="sdv")
                nc.scalar.activation(out=sdv, in_=var_g, func=Act.Sqrt, bias=eps8, scale=1.0)
                nc.vector.reciprocal(out=musig[:, 1:2], in_=sdv)
                # broadcast (mu, rstd) to channels; fold the gn affine:
                # a_c = gnw * rstd ; b_c = gnb - mu * a_c ; xn = a_c*x + b_c
                mps = ps.tile([128, 2], f32, tag="big", name=f"gnb_{i}")
                nc.tensor.matmul(out=mps, lhsT=indT, rhs=musig, start=True, stop=True)
                nc.vector.tensor_mul(af[:, i, 0:1], gnw_sb[:, i : i + 1], mps[:, 1:2])
                tmp = work.tile([128, 1], f32, tag="tmp1")
                nc.vector.tensor_mul(tmp, mps[:, 0:1], af[:, i, 0:1])
                nc.vector.tensor_sub(af[:, i, 1:2], gnb_sb[:, i : i + 1], tmp)
                # the affine itself runs on ACT (per-partition scale AND bias)
                nc.scalar.activation(
                    out=xn_sb[:, i, :],
                    in_=x_sb[:, i, :],
                    func=Act.Identity,
                    bias=af[:, i, 1:2],
                    scale=af[:, i, 0:1],
                )

            rhs3s = {}
            rhs3s[ARR[0]] = gn_stats(ARR[0])
            rhs3s[ARR[1]] = gn_stats(ARR[1])
            gn_finish(ARR[0], rhs3s[ARR[0]])
            rhs3s[ARR[2]] = gn_stats(ARR[2])
            gn_finish(ARR[1], rhs3s[ARR[1]])
            rhs3s[ARR[3]] = gn_stats(ARR[3])
            gn_finish(ARR[2], rhs3s[ARR[2]])
            gn_finish(ARR[3], rhs3s[ARR[3]])

            # ---------------- QKV building blocks ----------------
            q_sb = cst.tile([128, 4, T], bf16)
            k_sb = cst.tile([128, 4, T], bf16)
            vt_sb = cst.tile([128, 8, 8, 65], bf16)
            nc.vector.tensor_copy(
                out=vt_sb[:, :, :, 64:65],
                in_=nc.const_aps.tensor(1.0, (128, 8, 8, 1), bf16),
            )

            def emit_qk_half(j, n):
                qk_ps = ps.tile([128, 512], f32, tag="big", name=f"qk_{j}_{n}")
                for i in ARR:
                    nc.tensor.matmul(
                        out=qk_ps,
                        lhsT=w1t_sb[:, i, 128 * j : 128 * j + 128],
                        rhs=xn_sb[:, i, 512 * n : 512 * n + 512],
                        start=(i == ARR[0]),
                        stop=(i == ARR[3]),
                    )
                dst = q_sb if j < 4 else k_sb
                nc.vector.tensor_scalar(
                    out=dst[:, j % 4, 512 * n : 512 * n + 512],
                    in0=qk_ps,
                    scalar1=b1r_sb[:, j : j + 1],
                    scalar2=SCALE,
                    op0=Alu.add,
                    op1=Alu.mult,
                )

            def emit_v(st):
                vt_ps = ps.tile([128, 512], f32, tag="big", name=f"vt_{st}")
                for i in ARR:
                    nc.tensor.matmul(
                        out=vt_ps,
                        lhsT=xn_sb[:, i, 128 * st : 128 * st + 128],
                        rhs=w1t_sb[:, i, 2 * C : 3 * C],
                        start=(i == ARR[0]),
                        stop=(i == ARR[3]),
                    )
                nc.vector.scalar_tensor_tensor(
                    out=vt_sb[:, st, :, 0:64],
                    in0=vt_ps.rearrange("p (h c) -> p h c", c=64),
                    scalar=1.0,
                    in1=b1v_bc.rearrange("p (h c) -> p h c", c=64),
                    op0=Alu.mult,
                    op1=Alu.add,
                )

            # ---------------- attention building blocks ----------------
            wts = {}

            def emit_score_exp(p, st, hi):
                hp = 64 * hi
                st_ps = ps.tile([128, T], f32, tag="big", name=f"st_{p}_{st}_{hi}")
                for n in range(2):
                    nc.tensor.matmul(
                        out=st_ps[:, 512 * n : 512 * n + 512],
                        lhsT=k_sb[hp : hp + 64, p, 128 * st : 128 * st + 128],
                        rhs=q_sb[hp : hp + 64, p, 512 * n : 512 * n + 512],
                        start=True,
                        stop=True,
                        tile_position=(hp, 0),
                    )
                wt = wtp.tile([128, T], bf16, tag="wt", bufs=24, name=f"wt_{p}_{st}_{hi}")
                nc.scalar.activation(out=wt, in_=st_ps, func=Act.Exp, bias=0.0, scale=1.0)
                wts[(p, st, hi)] = wt

            av_tiles = {}

            def av_of(p):
                if p not in av_tiles:
                    av_tiles[p] = {
                        hi: ps.tile([128, T], f32, tag="av", name=f"av_{p}_{hi}")
                        for hi in range(2)
                    }
                return av_tiles[p]

            def emit_av(p, st):
                av = av_of(p)
                for hi in range(2):
                    h = 2 * p + hi
                    wt = wts.pop((p, st, hi))
                    for n in range(2):
                        nc.tensor.matmul(
                            out=av[hi][0:65, 512 * n : 512 * n + 512],
                            lhsT=vt_sb[:, st, h, 0:65],
                            rhs=wt[:, 512 * n : 512 * n + 512],
                            start=(st == 0),
                            stop=(st == 7),
                        )

            # epilogue in three stages, emitted steps apart so nothing sits
            # at a queue head waiting on a long-latency producer.
            epi = {}

            def emit_epi_a(p):
                av = av_tiles.pop(p)
                epi[p] = []
                for hi in range(2):
                    h = 2 * p + hi
                    o65 = wtp.tile([65, T], f32, tag="o65", bufs=3, name=f"o_{h}")
                    nc.vector.tensor_copy(out=o65, in_=av[hi][0:65, :])
                    rsp = wtp.tile([128, 8], f32, tag="rsp", bufs=3, name=f"rsp_{h}")
                    nc.sync.dma_start(out=rsp, in_=o65[64:65, :])
                    epi[p].append((o65, rsp))

            def emit_epi_b(p):
                for hi in range(2):
                    h = 2 * p + hi
                    o65, rsp = epi[p][hi]
                    rsp2 = wtp.tile([128, 8], f32, tag="rsp2", bufs=3, name=f"rsp2_{h}")
                    nc.vector.reciprocal(out=rsp2, in_=rsp)
                    rrow = wtp.tile([1, T], f32, tag="rrow", bufs=3, name=f"rrow_{h}")
                    nc.sync.dma_start(out=rrow, in_=rsp2)
                    rbc = wtp.tile([64, T], f32, tag="rb", bufs=3, name=f"rbc_{h}")
                    nc.gpsimd.partition_broadcast(rbc, rrow)
                    epi[p][hi] = (o65, rbc)

            def emit_epi_c(p):
                for hi in range(2):
                    h = 2 * p + hi
                    o65, rbc = epi[p][hi]
                    nc.vector.tensor_mul(o65[0:64, :], o65[0:64, :], rbc)
                    nc.vector.tensor_add(o65[0:64, :], o65[0:64, :], x_hd[:, h, :])
                    nc.sync.dma_start(out=out_d[64 * h : 64 * h + 64, :], in_=o65[0:64, :])
                del epi[p]

            # ---------------- the interleaved schedule ----------------
            emit_qk_half(0, 0)
            emit_qk_half(0, 1)
            emit_qk_half(4, 0)
            emit_qk_half(4, 1)

            units = {
                0: [lambda: emit_qk_half(1, 0), lambda: emit_qk_half(1, 1),
                    lambda: emit_qk_half(5, 0), lambda: emit_qk_half(5, 1),
                    lambda: emit_v(0), lambda: emit_v(1),
                    lambda: emit_v(2), lambda: emit_v(3)],
                1: [lambda: emit_v(4), lambda: emit_v(5),
                    lambda: emit_v(6), lambda: emit_v(7),
                    lambda: emit_qk_half(2, 0), lambda: emit_qk_half(2, 1),
                    lambda: emit_qk_half(6, 0), lambda: emit_qk_half(6, 1)],
                2: [lambda: emit_qk_half(3, 0), lambda: emit_qk_half(3, 1),
                    lambda: emit_qk_half(7, 0), lambda: emit_qk_half(7, 1),
                    None, None, None, None],
                3: [None] * 8,
            }
            av_sched = {
                0: {},
                1: {s: [(0, s)] for s in range(8)},
                2: {s: [(1, 2 * s), (1, 2 * s + 1)] for s in range(4)},
                3: {
                    **{s: [(2, 2 * s), (2, 2 * s + 1)] for s in range(4)},
                    5: [(3, 0), (3, 1)],
                    6: [(3, 2), (3, 3)],
                    7: [(3, 4), (3, 5)],
                },
            }
            # (pair, step) -> epilogue stage emissions
            epi_sched = {
                (2, 0): lambda: emit_epi_a(0),
                (2, 1): lambda: emit_epi_b(0),
                (2, 3): lambda: emit_epi_c(0),
                (2, 4): lambda: emit_epi_a(1),
                (2, 6): lambda: emit_epi_b(1),
                (3, 0): lambda: emit_epi_c(1),
                (3, 4): lambda: emit_epi_a(2),
                (3, 6): lambda: emit_epi_b(2),
            }

            for p in range(4):
                for s in range(8):
                    emit_score_exp(p, s, 0)
                    emit_score_exp(p, s, 1)
                    u = units[p][s]
                    if u is not None:
                        u()
                    for (pp, st) in av_sched[p].get(s, []):
                        emit_av(pp, st)
                    e = epi_sched.get((p, s))
                    if e is not None:
                        e()
            emit_epi_c(2)
            emit_av(3, 6)
            emit_av(3, 7)
            emit_epi_a(3)
            emit_epi_b(3)
            emit_epi_c(3)

    nc.finalize()
    return nc


def _make_in_maps(inputs):
    x = np.ascontiguousarray(np.asarray(inputs["x"], dtype=np.float32))
    gnw = np.asarray(inputs["gn_weight"], dtype=np.float32)
    gnb = np.asarray(inputs["gn_bias"], dtype=np.float32)
    w1 = np.asarray(inputs["w1"], dtype=np.float32)
    b1 = np.asarray(inputs["b1"], dtype=np.float32)

    import ml_dtypes

    B = x.shape[0]
    w1t = np.ascontiguousarray(w1[:, :, 0].T).astype(ml_dtypes.bfloat16)  # [C, 3C]
    b1r = np.ascontiguousarray(b1.reshape(12, 128).T)              # [128, 12]
    b1v = np.ascontiguousarray(b1[2 * C : 3 * C].reshape(1, C))    # [1, C]
    gnw_r = np.ascontiguousarray(gnw.reshape(4, 128).T)            # [128, 4]
    gnb_r = np.ascontiguousarray(gnb.reshape(4, 128).T)            # [128, 4]

    ind16 = np.zeros((128, 8), np.float32)
    indT = np.zeros((8, 128), np.float32)
    for g in range(8):
        ind16[16 * g : 16 * g + 16, g] = 1.0 / 16.0
        indT[g, 16 * g : 16 * g + 16] = 1.0

    in_maps = []
    for b in range(B):
        in_maps.append(
            {
                "x": np.ascontiguousarray(x[b].reshape(C, T)),
                "w1t": w1t,
                "b1r": b1r,
                "b1v": b1v,
                "gnw": gnw_r,
                "gnb": gnb_r,
                "ind16": ind16,
                "indT": indT,
            }
        )
    return in_maps


def _gather(results, x_shape):
    B, Cc, H, W = x_shape
    out = np.empty((B, Cc, H, W), dtype=np.float32)
    for b in range(B):
        out[b] = results[b]["out"].reshape(Cc, H, W)
    return out


def kernel(**inputs):
    from concourse.bass_utils import run_bass_kernel_spmd

    nc = _build_nc()
    in_maps = _make_in_maps(inputs)
    res = run_bass_kernel_spmd(nc, in_maps, core_ids=list(range(N_CORES)))
    return _gather(res.results, np.asarray(inputs["x"]).shape)


# revision 13
# speedup vs baseline: 1.2938x; 1.0090x over previous
"""Trainium2 Bass kernel for nn_Attention_64235530879146.

Reference computation (per batch element, C=512, T=H*W=1024, 32 groups,
8 heads of ch=64):
    xn = GroupNorm(x) * gn_weight + gn_bias          # [C, T]
    qkv = W1 @ xn + b1                               # [3C, T]
    per head: St[s,t] = (k*sc)^T (q*sc),  sc = ch**-0.25
              Wt = exp(St)   (no max subtraction; |S| < 8 for N(0,1) inputs,
                              far inside fp32 exp range)
              a[c,t] = sum_s v[c,s] Wt[s,t] / r[t],  r[t] = sum_s Wt[s,t]
    out = a + x
Sharding: pure data-parallel over batch - 8 batch elements on 8 NeuronCores,
no collectives.

The ScalarE exp stream (64 ACTIVATEs of [128,1024] = ~75 us with sem
overhead) is the hard floor for this problem. The schedule starts that
stream as early as possible and never lets it starve:

  - DMA: one queue carries ~100 GB/s, so x is split over the Sync and ACT
    HWDGE queues, and w1t is chunked in *consumption* order (q0/q1+k4/k5
    columns right after x, v and late head chunks after) so the first
    score matmul is gated by GroupNorm, not weights.
  - GroupNorm runs as four independent per-c-tile pipelines (the 16-channel
    groups never cross a 128-channel tile): bn_stats -> tiny PE group
    reduce -> Sqrt (ACT) -> DVE reciprocal -> PE broadcast -> fused affine.
    The affine xn = a_c*x + b_c runs on ACT (Identity with per-partition
    scale AND bias APs), which keeps the DVE chain off the critical path.
    Tiles are processed in DMA-arrival order (0,2,1,3 - sync/scalar queues
    interleave).
  - scores: per (pair, head, s-chunk) one [128,1024] f32 PSUM tile, 2
    N=512 matmuls; the two heads of a pair use tile_position (0,0)/(64,0)
    to run in disjoint PE row groups. ACT Exp PSUM->SBUF (bf16) FD=1024.
  - The remaining QKV projection chunks are interleaved one per attention
    step; AV (lhsT = vT_aug[128,65], ones column emits the softmax
    denominator r) runs with a one-pair lag inside the next pair's score
    stream, compressed 2-per-step from pair 2 so pair 3's own AV starts
    mid-pass and the tail stays short.
  - PSUM (8 banks): scores+projections+GN share a 2-slot x 2-bank pool;
    AV accumulators get 2 slots x 2 banks.
  - Epilogue per head is cut into three stages emitted several attention
    steps apart, so no DVE/queue head-of-line blocking: (a) copy [65,T]
    PSUM->SBUF (frees the AV slot) + DMA-reshape r to [128,8]; (b)
    all-lane DVE reciprocal + DMA back to a row + GpSimd
    partition_broadcast to the 64 channel lanes (a Q7 compute op - the
    DMA row-broadcast runs at ~25 GB/s and stalled the whole pipeline);
    (c) in-place o*=1/r, o+=x, store on the Sync queue.

Matmul inputs are bf16 (fp32 PSUM accumulate): measured end-to-end relative
error vs an fp64 reference is ~3.5e-4. Weights are transposed/reformatted on
the host in _make_in_maps (pure layout prep, no arithmetic beyond a bf16
cast).
"""
import numpy as np

GROUPS = 32
HEADS = 8
EPS = 1e-5
C = 512
T = 1024
CH = C // HEADS            # 64
SCALE = float(CH) ** -0.25
N_CORES = 8

# c-tile processing order = DMA arrival order (x0,x2 on sync; x1,x3 on gpsimd)
ARR = (0, 1, 2, 3)


def _build_nc():
    import concourse.bass as bass
    import concourse.mybir as mybir
    import concourse.tile as tile
    from concourse import bacc
    from concourse import library_config

    f32 = mybir.dt.float32
    bf16 = mybir.dt.bfloat16
    Alu = mybir.AluOpType
    Act = mybir.ActivationFunctionType

    nc = bacc.Bacc("TRN2", target_bir_lowering=False, debug=False)

    x_d = nc.declare_dram_parameter("x", [C, T], f32, isOutput=False)
    w1t_d = nc.declare_dram_parameter("w1t", [C, 3 * C], bf16, isOutput=False)
    b1r_d = nc.declare_dram_parameter("b1r", [128, 12], f32, isOutput=False)
    b1v_d = nc.declare_dram_parameter("b1v", [1, C], f32, isOutput=False)
    gnw_d = nc.declare_dram_parameter("gnw", [128, 4], f32, isOutput=False)
    gnb_d = nc.declare_dram_parameter("gnb", [128, 4], f32, isOutput=False)
    ind16_d = nc.declare_dram_parameter("ind16", [128, 8], f32, isOutput=False)
    indT_d = nc.declare_dram_parameter("indT", [8, 128], f32, isOutput=False)
    out_d = nc.declare_dram_parameter("out", [C, T], f32, isOutput=True)

    with tile.TileContext(nc) as tc:
        with (
            tc.tile_pool(name="const", bufs=1) as cst,
            tc.tile_pool(name="work", bufs=2) as work,
            tc.tile_pool(name="wtp", bufs=4) as wtp,
            tc.tile_pool(name="ps", bufs=2, space="PSUM") as ps,
        ):
            # partition_broadcast lives in the `attn` GpSimd library.
            nc.gpsimd.load_library(library_config.attn)

            # ---------------- loads ----------------
            # DMA issues on the ACT queue cost ~2.5 us each (vs ~0.6 us on
            # Sync/GpSimd), so NOTHING loads through the scalar queue - it
            # stays free for GroupNorm ACT ops and the exp stream. Loads are
            # split sync/gpsimd in consumption order.
            xv = x_d.ap().rearrange("(i p) t -> i p t", p=128)
            x_sb = cst.tile([128, 4, T], f32)
            w1t_sb = cst.tile([128, 4, 3 * C], bf16)
            w1tv = w1t_d.ap().rearrange("(i p) o -> p i o", p=128)

            def w1t_load(eng, lo, hi):
                eng.dma_start(out=w1t_sb[:, :, lo:hi], in_=w1tv[:, :, lo:hi])

            # gpsimd queue: tiny GN constants first, then its share of x/w1t
            ind16 = cst.tile([128, 8], f32)
            nc.gpsimd.dma_start(out=ind16, in_=ind16_d[:, :])
            indT = cst.tile([8, 128], f32)
            nc.gpsimd.dma_start(out=indT, in_=indT_d[:, :])
            gnw_sb = cst.tile([128, 4], f32)
            nc.gpsimd.dma_start(out=gnw_sb, in_=gnw_d[:, :])
            gnb_sb = cst.tile([128, 4], f32)
            nc.gpsimd.dma_start(out=gnb_sb, in_=gnb_d[:, :])
            nc.sync.dma_start(out=x_sb[:, 0, :], in_=xv[0])
            nc.gpsimd.dma_start(out=x_sb[:, 1, :], in_=xv[1])
            nc.sync.dma_start(out=x_sb[:, 2, :], in_=xv[2])
            nc.gpsimd.dma_start(out=x_sb[:, 3, :], in_=xv[3])
            b1r_sb = cst.tile([128, 12], f32)
            nc.gpsimd.dma_start(out=b1r_sb, in_=b1r_d[:, :])
            # q0/q1 and k4/k5 columns right behind x; v and late-head columns
            # follow on the gpsimd queue.
            w1t_load(nc.sync, 0, 256)
            w1t_load(nc.sync, 512, 768)
            b1v_bc = cst.tile([128, C], f32)
            nc.gpsimd.dma_start(out=b1v_bc, in_=b1v_d.ap().to_broadcast((128, C)))
            w1t_load(nc.gpsimd, 1024, 1536)
            w1t_load(nc.gpsimd, 256, 512)
            w1t_load(nc.gpsimd, 768, 1024)
            # head-aligned residual copy of x (needed only from ~45 us on)
            x_hd = cst.tile([64, 8, T], f32)
            nc.sync.dma_start(out=x_hd, in_=x_d.ap().rearrange("(h p) t -> p h t", p=64))
            eps8 = cst.tile([8, 1], f32)
            nc.vector.memset(eps8, EPS)

            # ---------------- GroupNorm: four per-tile pipelines ----------------
            # Groups are 16 channels, fully inside one 128-channel tile, so
            # each tile computes stats -> rstd -> affine independently and
            # feeds the QKV accumulation as soon as it's done.
            xn_sb = cst.tile([128, 4, T], bf16)
            af = cst.tile([128, 4, 2], f32)

            def gn_stats(i):
                st6 = work.tile([128, 2, 6], f32, tag="st6")
                nc.vector.bn_stats(out=st6[:, 0, :], in_=x_sb[:, i, 0:512])
                nc.vector.bn_stats(out=st6[:, 1, :], in_=x_sb[:, i, 512:1024])
                mv = work.tile([128, 2], f32, tag="mv")
                nc.vector.bn_aggr(out=mv, in_=st6)
                rhs3 = work.tile([128, 3], f32, tag="rhs3")
                nc.vector.tensor_copy(out=rhs3[:, 0:2], in_=mv)
                nc.vector.tensor_mul(rhs3[:, 2:3], mv[:, 0:1], mv[:, 0:1])
                return rhs3

            def gn_finish(i, rhs3):
                # group reduce: [8, (mu, Evar, Emu2)] for this tile's 8 groups
                sps = ps.tile([8, 3], f32, tag="big", name=f"gn_{i}")
                nc.tensor.matmul(out=sps, lhsT=ind16, rhs=rhs3, start=True, stop=True)
                sg = work.tile([8, 3], f32, tag="sg")
                nc.vector.tensor_copy(out=sg, in_=sps)
                musig = work.tile([8, 2], f32, tag="musig")
                nc.vector.tensor_copy(out=musig[:, 0:1], in_=sg[:, 0:1])
                var_g = work.tile([8, 1], f32, tag="varg")
                nc.vector.tensor_add(var_g, sg[:, 1:2], sg[:, 2:3])
                mu2 = work.tile([8, 1], f32, tag="mu2")
                nc.vector.tensor_mul(mu2, sg[:, 0:1], sg[:, 0:1])
                nc.vector.tensor_sub(var_g, var_g, mu2)
                sdv = work.tile([8, 1], f32, tag="sdv")
                nc.scalar.activation(out=sdv, in_=var_g, func=Act.Sqrt, bias=eps8, scale=1.0)
                nc.vector.reciprocal(out=musig[:, 1:2], in_=sdv)
                # broadcast (mu, rstd) to channels; fold the gn affine:
                # a_c = gnw * rstd ; b_c = gnb - mu * a_c ; xn = a_c*x + b_c
                mps = ps.tile([128, 2], f32, tag="big", name=f"gnb_{i}")
                nc.tensor.matmul(out=mps, lhsT=indT, rhs=musig, start=True, stop=True)
                nc.vector.tensor_mul(af[:, i, 0:1], gnw_sb[:, i : i + 1], mps[:, 1:2])
                tmp = work.tile([128, 1], f32, tag="tmp1")
                nc.vector.tensor_mul(tmp, mps[:, 0:1], af[:, i, 0:1])
                nc.vector.tensor_sub(af[:, i, 1:2], gnb_sb[:, i : i + 1], tmp)
                # the affine itself runs on ACT (per-partition scale AND bias)
                nc.scalar.activation(
                    out=xn_sb[:, i, :],
                    in_=x_sb[:, i, :],
                    func=Act.Identity,
                    bias=af[:, i, 1:2],
                    scale=af[:, i, 0:1],
                )

            rhs3s = {}
            rhs3s[ARR[0]] = gn_stats(ARR[0])
            rhs3s[ARR[1]] = gn_stats(ARR[1])
            gn_finish(ARR[0], rhs3s[ARR[0]])
            rhs3s[ARR[2]] = gn_stats(ARR[2])
            gn_finish(ARR[1], rhs3s[ARR[1]])
            rhs3s[ARR[3]] = gn_stats(ARR[3])
            gn_finish(ARR[2], rhs3s[ARR[2]])
            gn_finish(ARR[3], rhs3s[ARR[3]])

            # ---------------- QKV building blocks ----------------
            q_sb = cst.tile([128, 4, T], bf16)
            k_sb = cst.tile([128, 4, T], bf16)
            vt_sb = cst.tile([128, 8, 8, 65], bf16)
            nc.vector.tensor_copy(
                out=vt_sb[:, :, :, 64:65],
                in_=nc.const_aps.tensor(1.0, (128, 8, 8, 1), bf16),
            )

            def emit_qk_half(j, n):
                qk_ps = ps.tile([128, 512], f32, tag="big", name=f"qk_{j}_{n}")
                for i in ARR:
                    nc.tensor.matmul(
                        out=qk_ps,
                        lhsT=w1t_sb[:, i, 128 * j : 128 * j + 128],
                        rhs=xn_sb[:, i, 512 * n : 512 * n + 512],
                        start=(i == ARR[0]),
                        stop=(i == ARR[3]),
                    )
                dst = q_sb if j < 4 else k_sb
                nc.vector.tensor_scalar(
                    out=dst[:, j % 4, 512 * n : 512 * n + 512],
                    in0=qk_ps,
                    scalar1=b1r_sb[:, j : j + 1],
                    scalar2=SCALE,
                    op0=Alu.add,
                    op1=Alu.mult,
                )

            def emit_v(st):
                vt_ps = ps.tile([128, 512], f32, tag="big", name=f"vt_{st}")
                for i in ARR:
                    nc.tensor.matmul(
                        out=vt_ps,
                        lhsT=xn_sb[:, i, 128 * st : 128 * st + 128],
                        rhs=w1t_sb[:, i, 2 * C : 3 * C],
                        start=(i == ARR[0]),
                        stop=(i == ARR[3]),
                    )
                nc.vector.scalar_tensor_tensor(
                    out=vt_sb[:, st, :, 0:64],
                    in0=vt_ps.rearrange("p (h c) -> p h c", c=64),
                    scalar=1.0,
                    in1=b1v_bc.rearrange("p (h c) -> p h c", c=64),
                    op0=Alu.mult,
                    op1=Alu.add,
                )

            # ---------------- attention building blocks ----------------
            wts = {}

            def emit_score_exp(p, st, hi):
                hp = 64 * hi
                st_ps = ps.tile([128, T], f32, tag="big", name=f"st_{p}_{st}_{hi}")
                for n in range(2):
                    nc.tensor.matmul(
                        out=st_ps[:, 512 * n : 512 * n + 512],
                        lhsT=k_sb[hp : hp + 64, p, 128 * st : 128 * st + 128],
                        rhs=q_sb[hp : hp + 64, p, 512 * n : 512 * n + 512],
                        start=True,
                        stop=True,
                        tile_position=(hp, 0),
                    )
                wt = wtp.tile([128, T], bf16, tag="wt", bufs=24, name=f"wt_{p}_{st}_{hi}")
                nc.scalar.activation(out=wt, in_=st_ps, func=Act.Exp, bias=0.0, scale=1.0)
                wts[(p, st, hi)] = wt

            av_tiles = {}

            def av_of(p):
                if p not in av_tiles:
                    av_tiles[p] = {
                        hi: ps.tile([128, T], f32, tag="av", name=f"av_{p}_{hi}")
                        for hi in range(2)
                    }
                return av_tiles[p]

            def emit_av_h(p, st, hi):
                av = av_of(p)
                h = 2 * p + hi
                wt = wts.pop((p, st, hi))
                for n in range(2):
                    nc.tensor.matmul(
                        out=av[hi][0:65, 512 * n : 512 * n + 512],
                        lhsT=vt_sb[:, st, h, 0:65],
                        rhs=wt[:, 512 * n : 512 * n + 512],
                        start=(st == 0),
                        stop=(st == 7),
                    )

            def emit_av(p, st):
                emit_av_h(p, st, 0)
                emit_av_h(p, st, 1)

            # epilogue in three per-head stages, emitted steps apart so
            # nothing sits at a queue head waiting on a long-latency producer.
            epi = {}

            def emit_epi_a(p, hi):
                h = 2 * p + hi
                av = av_tiles[p].pop(hi)
                if not av_tiles[p]:
                    del av_tiles[p]
                o65 = wtp.tile([65, T], f32, tag="o65", bufs=3, name=f"o_{h}")
                nc.vector.tensor_copy(out=o65, in_=av[0:65, :])
                rsp = wtp.tile([128, 8], f32, tag="rsp", bufs=3, name=f"rsp_{h}")
                nc.sync.dma_start(out=rsp, in_=o65[64:65, :])
                epi[h] = (o65, rsp)

            def emit_epi_b(p, hi):
                h = 2 * p + hi
                o65, rsp = epi[h]
                rsp2 = wtp.tile([128, 8], f32, tag="rsp2", bufs=3, name=f"rsp2_{h}")
                nc.vector.reciprocal(out=rsp2, in_=rsp)
                rrow = wtp.tile([1, T], f32, tag="rrow", bufs=3, name=f"rrow_{h}")
                nc.sync.dma_start(out=rrow, in_=rsp2)
                rbc = wtp.tile([64, T], f32, tag="rb", bufs=3, name=f"rbc_{h}")
                nc.gpsimd.partition_broadcast(rbc, rrow)
                epi[h] = (o65, rbc)

            def emit_epi_c(p, hi):
                h = 2 * p + hi
                o65, rbc = epi.pop(h)
                nc.vector.tensor_mul(o65[0:64, :], o65[0:64, :], rbc)
                nc.vector.tensor_add(o65[0:64, :], o65[0:64, :], x_hd[:, h, :])
                nc.sync.dma_start(out=out_d[64 * h : 64 * h + 64, :], in_=o65[0:64, :])

            # ---------------- the interleaved schedule ----------------
            emit_qk_half(0, 0)
            emit_qk_half(0, 1)
            emit_qk_half(4, 0)
            emit_qk_half(4, 1)

            units = {
                0: [lambda: emit_qk_half(1, 0), lambda: emit_qk_half(1, 1),
                    lambda: emit_qk_half(5, 0), lambda: emit_qk_half(5, 1),
                    lambda: emit_v(0), lambda: emit_v(1),
                    lambda: emit_v(2), lambda: emit_v(3)],
                1: [lambda: emit_v(4), lambda: emit_v(5),
                    lambda: emit_v(6), lambda: emit_v(7),
                    lambda: emit_qk_half(2, 0), lambda: emit_qk_half(2, 1),
                    lambda: emit_qk_half(6, 0), lambda: emit_qk_half(6, 1)],
                2: [lambda: emit_qk_half(3, 0), lambda: emit_qk_half(3, 1),
                    lambda: emit_qk_half(7, 0), lambda: emit_qk_half(7, 1),
                    None, None, None, None],
                3: [None] * 8,
            }
            # AV placement (pp, st, hi): one-pair lag, compressed 2-per-step
            # from pair 2 on; pair 3's head-0 AV runs inside its own pass so
            # its epilogue chain starts before the last exp.
            AB = (0, 1)
            av_sched = {
                0: {},
                1: {s: [(0, s, 0), (0, s, 1)] for s in range(8)},
                2: {s: [(1, 2 * s, hi) for hi in AB] + [(1, 2 * s + 1, hi) for hi in AB]
                    for s in range(4)},
                3: {
                    **{s: [(2, 2 * s, hi) for hi in AB] + [(2, 2 * s + 1, hi) for hi in AB]
                       for s in range(4)},
                    5: [(3, 0, 0), (3, 1, 0), (3, 2, 0), (3, 3, 0)],
                    6: [(3, 4, 0), (3, 5, 0), (3, 0, 1), (3, 1, 1)],
                    7: [(3, 6, 0), (3, 7, 0)],
                },
            }
            # (pair, step) -> epilogue stage emissions
            epi_sched = {
                (2, 0): lambda: (emit_epi_a(0, 0), emit_epi_a(0, 1)),
                (2, 1): lambda: (emit_epi_b(0, 0), emit_epi_b(0, 1)),
                (2, 3): lambda: (emit_epi_c(0, 0), emit_epi_c(0, 1)),
                (2, 4): lambda: (emit_epi_a(1, 0), emit_epi_a(1, 1)),
                (2, 6): lambda: (emit_epi_b(1, 0), emit_epi_b(1, 1)),
                (3, 0): lambda: (emit_epi_c(1, 0), emit_epi_c(1, 1)),
                (3, 4): lambda: (emit_epi_a(2, 0), emit_epi_a(2, 1)),
                (3, 6): lambda: (emit_epi_b(2, 0), emit_epi_b(2, 1)),
            }

            for p in range(4):
                for s in range(8):
                    emit_score_exp(p, s, 0)
                    emit_score_exp(p, s, 1)
                    u = units[p][s]
                    if u is not None:
                        u()
                    for (pp, st, hi) in av_sched[p].get(s, []):
                        emit_av_h(pp, st, hi)
                    e = epi_sched.get((p, s))
                    if e is not None:
                        e()
            # tail: head 6's AV is complete - its epilogue chain overlaps
            # head 7's remaining AV matmuls and both chains pipeline out.
            emit_epi_c(2, 0)
            emit_epi_c(2, 1)
            emit_epi_a(3, 0)
            for st in (2, 3, 4, 5):
                emit_av_h(3, st, 1)
            emit_epi_b(3, 0)
            emit_av_h(3, 6, 1)
            emit_av_h(3, 7, 1)
            emit_epi_a(3, 1)
            emit_epi_c(3, 0)
            emit_epi_b(3, 1)
            emit_epi_c(3, 1)

    nc.finalize()
    return nc


def _make_in_maps(inputs):
    x = np.ascontiguousarray(np.asarray(inputs["x"], dtype=np.float32))
    gnw = np.asarray(inputs["gn_weight"], dtype=np.float32)
    gnb = np.asarray(inputs["gn_bias"], dtype=np.float32)
    w1 = np.asarray(inputs["w1"], dtype=np.float32)
    b1 = np.asarray(inputs["b1"], dtype=np.float32)

    import ml_dtypes

    B = x.shape[0]
    w1t = np.ascontiguousarray(w1[:, :, 0].T).astype(ml_dtypes.bfloat16)  # [C, 3C]
    b1r = np.ascontiguousarray(b1.reshape(12, 128).T)              # [128, 12]
    b1v = np.ascontiguousarray(b1[2 * C : 3 * C].reshape(1, C))    # [1, C]
    gnw_r = np.ascontiguousarray(gnw.reshape(4, 128).T)            # [128, 4]
    gnb_r = np.ascontiguousarray(gnb.reshape(4, 128).T)            # [128, 4]

    ind16 = np.zeros((128, 8), np.float32)
    indT = np.zeros((8, 128), np.float32)
    for g in range(8):
        ind16[16 * g : 16 * g + 16, g] = 1.0 / 16.0
        indT[g, 16 * g : 16 * g + 16] = 1.0

    in_maps = []
    for b in range(B):
        in_maps.append(
            {
                "x": np.ascontiguousarray(x[b].reshape(C, T)),
                "w1t": w1t,
                "b1r": b1r,
                "b1v": b1v,
                "gnw": gnw_r,
                "gnb": gnb_r,
                "ind16": ind16,
                "indT": indT,
            }
        )
    return in_maps


def _gather(results, x_shape):
    B, Cc, H, W = x_shape
    out = np.empty((B, Cc, H, W), dtype=np.float32)
    for b in range(B):
        out[b] = results[b]["out"].reshape(Cc, H, W)
    return out


def kernel(**inputs):
    from concourse.bass_utils import run_bass_kernel_spmd

    nc = _build_nc()
    in_maps = _make_in_maps(inputs)
    res = run_bass_kernel_spmd(nc, in_maps, core_ids=list(range(N_CORES)))
    return _gather(res.results, np.asarray(inputs["x"]).shape)
